# revision 24
# baseline (speedup 1.0000x reference)
"""Triangular matmul C = triu(triu(A) @ triu(B)) on 8 TRN2 NeuronCores.

Design: 1D row-parallel over 128x128 blocks (N=4096 -> 32 blocks/side).
The (I, K, J) block-tetrahedron {I <= K <= J} is sharded by output row-block
I across the 8 cores.  Each core keeps its whole working set of B resident
in SBUF: the packed upper-triangular strip suffix {B[K,128K:] : K >= minI}
is at most 132 KB/partition (16.9 MB) and is streamed from HBM exactly once
per core.  A^T strips (one 128x128 block per (I,K) pair) are host-packed
per core and loaded once (<22 KB/partition).

Numerics: single-pass bf16 (hi plane only).  Host-measured rel-err vs the
fp32 reference is ~2.4e-3 (4.4e-3 with the bf16 output store) against a
2e-2 harness gate.  PE work is 1 matmul pass (vs 3 for the baseline's
bf16x3 split): ~40 us of matmul columns per core at 2.4 GHz.

Schedule per core: the lowest row ("chase" row) runs K-major, consuming the
ascending-K B stream as strips arrive; its PSUM bank c (J-window
[512c,512c+512)) closes at strip K == 4c+3 and is evicted immediately.  At
that point every strip the other ("resident") rows' bank-c chains need is
in SBUF, so those chains are emitted right there, bank-major, filling the
chase's DMA-wait gaps with a PE instruction order that stays consumable
under real DMA pacing.  Resident chains rotate over the chase-vacated
physical PSUM banks {0..c} so each chain's eviction copy (DVE fp32->bf16)
hides behind the next chains' matmuls instead of stalling the PE on a
same-bank WAR.  C stores are deferred to SBUF staging until the B stream
finishes, so they do not steal HBM bandwidth from the chase; they flush
over the post-chase bank-7 chains.

Cores are balanced with a measured cost model (~92 ns/block DMA-paced
chase, 53.3 ns/block matmul + ~35-53 ns/matmul exposed LDWEIGHTS, which
tile_legalize emits per matmul), with minI spread 0..7 to minimize the
duplicated B traffic and BINS ordered so HBM-domain pairs (adjacent device
indices, ~716 GB/s shared) couple a heavy B-stream core with a light one.

The kernel takes FULL (unsharded) inputs and returns the FULL output.
"""

import numpy as np

N = 4096
BLK = 128
NB = N // BLK  # 32
N_CORES = 8
PHASE = 512  # J-chunk width (one PSUM bank of fp32)
MODE = "bf16x1"

T = lambda m: m * (m + 1) // 2

# Row-block -> core assignment, balanced for (B-chase DMA + compute) cost;
# minI spread 0..7 so the duplicated B traffic (T(32-minI) blocks/core) is
# minimized while work stays ~equal.  Order pairs heavy B-streams (low minI)
# with light ones: HBM domains are shared per NeuronCore pair (~716 GB/s),
# so adjacent devices should not both stream ~17 MB.
BINS = [
    [0],
    [7, 8, 15, 27],
    [1, 14, 22],
    [6, 9, 16, 25, 30],
    [2, 13, 19, 26],
    [5, 10, 18, 23, 28],
    [3, 12, 20, 21, 29],
    [4, 11, 17, 24, 31],
]
MAXB = max(len(b) for b in BINS)  # output row-slots per core
NSLOT = max(sum(NB - I for I in b) for b in BINS)  # (I,K)-pairs max/core

# bcat layout: strips ascending K; strip K is B[128K:128K+128, 128K:] with
# (32-K)*128 cols.  BOFF[K] = column offset of strip K.
BTOT = T(NB) * BLK  # 67584 cols
BOFF = {K: BLK * (T(NB) - T(NB - K)) for K in range(NB)}

# apack layout per core: rows ascending, then K = I..31; one 128x128 block
# (A^T[K-strip, I-cols]) per slot.
ASLOT = []
for _b in BINS:
    _d, _s = {}, 0
    for _I in sorted(_b):
        _d[_I] = _s
        _s += NB - _I
    ASLOT.append(_d)

BCHUNK = 2048  # B-stream DMA chunk (cols); 2048 cols * 2B = 0.5 MB per DMA
ACHUNK = 8  # A-load DMA granularity in k-blocks (0.25 MB per DMA)
N_WARM = 16  # HAM warmup matmuls (N=512 each, ~3.4 us of PE activity)


def _emit_core(nc, tc, pools, dram_io, core, variant="full"):
    import concourse.mybir as mybir

    f32 = mybir.dt.float32
    bf16 = mybir.dt.bfloat16
    apool, bpool, cpool, psum_pool, wpool = pools
    apack, bcat, cpart = dram_io["apack"], dram_io["bcat"], dram_io["cpart"]

    rows = sorted(BINS[core])
    mI = rows[0]
    bsuf = BTOT - BOFF[mI]  # resident B cols for this core

    if "empty" in variant:
        # Loop-overhead diagnostic: minimal body.
        z = wpool.tile([BLK, BLK], bf16, name="z", tag="warm", bufs=1)
        nc.gpsimd.memset(z[:], 0.0)
        return

    b_t = bpool.tile([BLK, bsuf], bf16, name="bres", tag="bres", bufs=1)
    a_t = apool.tile([BLK, NSLOT * BLK], bf16, name="a", tag="a", bufs=1)

    # HAM warmup: keep the PE busy while the first DMA chunks land so the
    # clock gate opens (4/8 -> 8/8) before real matmuls start.  Reads
    # uninitialized SBUF, writes a PSUM bank that row processing will
    # re-open with start=True.
    if "nomm" not in variant:
        wt = wpool.tile([BLK, BLK + PHASE], bf16, name="warm", tag="warm", bufs=1)
        nc.gpsimd.memset(wt[:], 0.0)
        wps = psum_pool.tile([BLK, PHASE], f32, name="wps", tag="ps7", bufs=1)
        for _ in range(N_WARM):
            nc.tensor.matmul(
                wps[:, :], wt[:, :BLK], wt[:, BLK:], start=True, stop=True
            )

    # B stream: ascending, 1 MB chunks, alternating the two HWDGE rings.
    if "nob" not in variant:
        qs = [nc.sync, nc.scalar]
        for qi, cur in enumerate(range(BOFF[mI], BTOT, BCHUNK)):
            end = min(cur + BCHUNK, BTOT)
            qs[qi % 2].dma_start(
                b_t[:, cur - BOFF[mI] : end - BOFF[mI]], bcat[:, cur:end]
            )
    else:
        # PE-only ablation: matmuls read a small memset window instead.
        nc.gpsimd.memset(b_t[:, :PHASE], 0.0)

    # A stream: per-row chunks on the SWDGE ring, in consumption order.
    for I in rows:
        base = ASLOT[core][I]
        for j0 in range(0, NB - I, ACHUNK):
            j1 = min(j0 + ACHUNK, NB - I)
            nc.gpsimd.dma_start(
                a_t[:, (base + j0) * BLK : (base + j1) * BLK],
                apack[:, (base + j0) * BLK : (base + j1) * BLK],
            )

    slot_row = {I: i for i, I in enumerate(rows)}

    def mm(ps_c, I, K, c, base):
        a_w = a_t[:, (base + K - I) * BLK : (base + K - I + 1) * BLK]
        cstart = max(PHASE * c, K * BLK)  # global col of this MM
        w = PHASE * (c + 1) - cstart
        boff = 0 if "nob" in variant else BOFF[K] - BOFF[mI] + cstart - K * BLK
        nc.tensor.matmul(
            ps_c[:, cstart - PHASE * c : PHASE],
            a_w,
            b_t[:, boff : boff + w],
            start=(K == I),
            stop=(K == min(4 * c + 3, NB - 1)),
        )

    pending = []

    def evict(ps_c, I, c, defer=False):
        if "noevict" in variant:
            return
        # Copy frees the PSUM bank immediately; the HBM store can be
        # deferred so it does not steal bandwidth from the B stream.
        coff0 = max(I * BLK - PHASE * c, 0)
        wv = PHASE - coff0
        ct = cpool.tile([BLK, PHASE], bf16, name=f"c_{I}_{c}", tag="cst", bufs=28)
        nc.vector.tensor_copy(ct[:, :wv], ps_c[:, coff0:PHASE])
        if defer:
            pending.append((I, c, ct, coff0, wv))
        else:
            r0 = slot_row[I] * BLK
            nc.gpsimd.dma_start(
                cpart[r0 : r0 + BLK, PHASE * c + coff0 : PHASE * (c + 1)], ct[:, :wv]
            )

    def flush_stores():
        # HWDGE rings only: SWDGE (gpsimd) descriptor generation can be
        # locked out of the shared SBUF port pair by the concurrent DVE
        # eviction copies of the post-chase bank-7 chains.
        sqs = [nc.sync, nc.scalar]
        for i, (I, c, ct, coff0, wv) in enumerate(pending):
            r0 = slot_row[I] * BLK
            sqs[i % 2].dma_start(
                cpart[r0 : r0 + BLK, PHASE * c + coff0 : PHASE * (c + 1)], ct[:, :wv]
            )
        pending.clear()

    if "nomm" in variant:
        return

    if "kmajor" in variant:
        # Fully serial reference schedule: each row K-major, rows in order.
        for I in rows:
            base = ASLOT[core][I]
            ps = {
                c: psum_pool.tile(
                    [BLK, PHASE], f32, name=f"ps_{I}_{c}", tag=f"ps{c}", bufs=1
                )
                for c in range(I // 4, NB // 4)
            }
            for K in range(I, NB):
                for c in range(K // 4, NB // 4):
                    mm(ps[c], I, K, c, base)
                if K % 4 == 3 and K // 4 >= I // 4:
                    evict(ps[K // 4], I, K // 4)
        return

    # Chase row (lowest I): K-major, consuming B strips as they arrive.
    # Bank c closes at K == 4c+3 and is evicted immediately; at that point
    # every strip a resident row's bank-c chain needs (K' <= 4c+3) is
    # already in SBUF, so those chains are emitted right there -- the PE
    # instruction order stays consumable under real DMA pacing and the
    # chase's DMA-wait gaps are filled with resident-row matmuls.
    I0 = rows[0]
    base0 = ASLOT[core][I0]
    ps = {
        c: psum_pool.tile([BLK, PHASE], f32, name=f"ps_{I0}_{c}", tag=f"ps{c}", bufs=1)
        for c in range(I0 // 4, NB // 4)
    }
    rot = 0  # physical-PSUM-bank rotation for resident chains
    for K in range(I0, NB):
        for c in range(K // 4, NB // 4):
            mm(ps[c], I0, K, c, base0)
        if K % 4 == 3:
            c = K // 4
            last = K == NB - 1
            evict(ps[c], I0, c, defer=not last)
            if last:
                # B stream is fully consumed: flush the deferred C stores
                # (they overlap the remaining bank-7 resident chains).
                flush_stores()
            kend = min(4 * c + 3, NB - 1)
            for R in rows[1:]:
                if R // 4 > c:
                    continue
                # Chase windows <= c are closed, so physical banks 0..c are
                # free; rotate chains across them so each chain's eviction
                # copy hides behind the next chains' matmuls instead of
                # stalling the PE on a same-bank WAR.
                b = rot % (c + 1)
                rot += 1
                ps_c = psum_pool.tile(
                    [BLK, PHASE], f32, name=f"ps_{R}_{c}", tag=f"ps{b}", bufs=1
                )
                for Kr in range(R, kend + 1):
                    mm(ps_c, R, Kr, c, base=ASLOT[core][R])
                evict(ps_c, R, c, defer=not last)
            if last:
                flush_stores()


def _build(mode=MODE, repeat=1, variant="full"):
    import concourse.mybir as mybir
    import concourse.tile as tile
    from concourse import bacc

    nc = bacc.Bacc(None, target_bir_lowering=False, debug=False)
    f32 = mybir.dt.float32
    bf16 = mybir.dt.bfloat16
    with tile.TileContext(nc) as tc:
        with (
            tc.tile_pool(name="dram", bufs=1, space="DRAM") as dram,
            tc.tile_pool(name="apool", bufs=1) as apool,
            tc.tile_pool(name="bpool", bufs=1) as bpool,
            tc.tile_pool(name="cpool", bufs=1) as cpool,
            tc.tile_pool(name="wpool", bufs=1) as wpool,
            tc.tile_pool(name="psum", bufs=1, space="PSUM") as psum_pool,
        ):
            dram_io = {
                "apack": dram.tile(
                    [BLK, NSLOT * BLK], bf16, kind="ExternalInput",
                    name="apack", uniquify=False,
                ),
                "bcat": dram.tile(
                    [BLK, BTOT], bf16, kind="ExternalInput",
                    name="bcat", uniquify=False,
                ),
                "cpart": dram.tile(
                    [MAXB * BLK, N], bf16, kind="ExternalOutput",
                    name="cpart", uniquify=False,
                ),
            }
            pid = nc.partition_id()
            pools = (apool, bpool, cpool, psum_pool, wpool)
            for c in range(N_CORES):
                cvar = variant
                if variant.startswith("solo"):
                    cvar = "full" if c == int(variant[4:]) else "empty"
                with tc.If(pid == c):
                    if repeat > 1:
                        with tc.For_i(
                            0, repeat, 1, hint_engines=tuple(mybir.ALL_ENGINES)
                        ):
                            _emit_core(nc, tc, pools, dram_io, c, cvar)
                    else:
                        _emit_core(nc, tc, pools, dram_io, c, cvar)
    nc.compile()
    return nc


_cached_nc = {}


def _get_nc(key=("full", 1)):
    if key not in _cached_nc:
        variant, repeat = key
        _cached_nc[key] = _build(repeat=repeat, variant=variant)
    return _cached_nc[key]


def _host_pack(A, B):
    import ml_dtypes

    bf16 = ml_dtypes.bfloat16
    AT = np.ascontiguousarray(A.T).astype(bf16)
    Bh = B.astype(bf16)
    bcat = np.empty((BLK, BTOT), dtype=bf16)
    for K in range(NB):
        bcat[:, BOFF[K] : BOFF[K] + (NB - K) * BLK] = Bh[K * BLK : (K + 1) * BLK, K * BLK :]
    apacks = []
    for c in range(N_CORES):
        ap = np.zeros((BLK, NSLOT * BLK), dtype=bf16)
        for I in sorted(BINS[c]):
            base = ASLOT[c][I]
            ap[:, base * BLK : (base + NB - I) * BLK] = np.concatenate(
                [AT[K * BLK : (K + 1) * BLK, I * BLK : (I + 1) * BLK] for K in range(I, NB)],
                axis=1,
            )
        apacks.append(ap)
    return apacks, bcat


LAST_RESULTS = None


def kernel(A, B):
    global LAST_RESULTS
    from concourse.bass_utils import run_bass_kernel_spmd

    A = np.asarray(A, dtype=np.float32)
    B = np.asarray(B, dtype=np.float32)
    nc = _get_nc()
    apacks, bcat = _host_pack(A, B)
    in_maps = [{"apack": apacks[c], "bcat": bcat} for c in range(N_CORES)]
    res = run_bass_kernel_spmd(nc, in_maps, core_ids=list(range(N_CORES)))
    LAST_RESULTS = res

    C = np.zeros((N, N), dtype=np.float32)
    for c in range(N_CORES):
        cp = res.results[c]["cpart"]
        for s, I in enumerate(sorted(BINS[c])):
            C[I * BLK : (I + 1) * BLK, I * BLK :] = cp[
                s * BLK : (s + 1) * BLK, I * BLK :
            ].astype(np.float32)
    return C


# revision 25
# speedup vs baseline: 1.1273x; 1.1273x over previous
"""Triangular matmul C = triu(triu(A) @ triu(B)) on 8 TRN2 NeuronCores.

Design: 1D row-parallel over 128x128 blocks (N=4096 -> 32 blocks/side).
The (I, K, J) block-tetrahedron {I <= K <= J} is sharded by output row-block
I across the 8 cores.  Each core keeps its whole working set of B resident
in SBUF: the packed upper-triangular strip suffix {B[K,128K:] : K >= minI}
is at most 132 KB/partition (16.9 MB) and is streamed from HBM exactly once
per core.  A^T strips (one 128x128 block per (I,K) pair) are host-packed
per core and loaded once (<22 KB/partition).

Numerics: single-pass bf16 (hi plane only).  Host-measured rel-err vs the
fp32 reference is ~2.4e-3 (4.4e-3 with the bf16 output store) against a
2e-2 harness gate.  PE work is 1 matmul pass (vs 3 for the baseline's
bf16x3 split): ~40 us of matmul columns per core at 2.4 GHz.

Schedule per core: the lowest row ("chase" row) runs K-major, consuming the
ascending-K B stream as strips arrive; its PSUM bank c (J-window
[512c,512c+512)) closes at strip K == 4c+3 and is evicted immediately.  At
that point every strip the other ("resident") rows' bank-c chains need is
in SBUF, so those chains are emitted right there, bank-major, filling the
chase's DMA-wait gaps with a PE instruction order that stays consumable
under real DMA pacing.  Resident chains rotate over the chase-vacated
physical PSUM banks {0..c} so each chain's eviction copy (DVE fp32->bf16)
hides behind the next chains' matmuls instead of stalling the PE on a
same-bank WAR.  C stores are deferred to SBUF staging until the B stream
finishes, so they do not steal HBM bandwidth from the chase; they flush
over the post-chase bank-7 chains.

Cores are balanced with a measured cost model (~92 ns/block DMA-paced
chase, 53.3 ns/block matmul + ~35-53 ns/matmul exposed LDWEIGHTS, which
tile_legalize emits per matmul), with minI spread 0..7 to minimize the
duplicated B traffic and BINS ordered so HBM-domain pairs (adjacent device
indices, ~716 GB/s shared) couple a heavy B-stream core with a light one.

The kernel takes FULL (unsharded) inputs and returns the FULL output.
"""

import numpy as np

N = 4096
BLK = 128
NB = N // BLK  # 32
N_CORES = 8
PHASE = 512  # J-chunk width (one PSUM bank of fp32)
MODE = "bf16x1"

T = lambda m: m * (m + 1) // 2

# Row-block -> core assignment, balanced for (B-chase DMA + compute) cost;
# minI spread 0..7 so the duplicated B traffic (T(32-minI) blocks/core) is
# minimized while work stays ~equal.  Order pairs heavy B-streams (low minI)
# with light ones: HBM domains are shared per NeuronCore pair (~716 GB/s),
# so adjacent devices should not both stream ~17 MB.
BINS = [
    [0, 17],
    [7, 12, 18, 21, 25, 27, 28, 31],
    [1, 9],
    [6, 11, 14, 30],
    [2, 15, 16],
    [5, 10, 20, 24, 26],
    [3, 13, 19, 23],
    [4, 8, 22, 29],
]
MAXB = max(len(b) for b in BINS)  # output row-slots per core
NSLOT = max(sum(NB - I for I in b) for b in BINS)  # (I,K)-pairs max/core

# bcat layout: strips ascending K; strip K is B[128K:128K+128, 128K:] with
# (32-K)*128 cols.  BOFF[K] = column offset of strip K.
BTOT = T(NB) * BLK  # 67584 cols
BOFF = {K: BLK * (T(NB) - T(NB - K)) for K in range(NB)}

# apack layout per core: rows ascending, then K = I..31; one 128x128 block
# (A^T[K-strip, I-cols]) per slot.
ASLOT = []
for _b in BINS:
    _d, _s = {}, 0
    for _I in sorted(_b):
        _d[_I] = _s
        _s += NB - _I
    ASLOT.append(_d)

BCHUNK = 2048  # B-stream DMA chunk (cols); 2048 cols * 2B = 0.5 MB per DMA
ACHUNK = 8  # A-load DMA granularity in k-blocks (0.25 MB per DMA)
N_WARM = 16  # HAM warmup matmuls (N=512 each, ~3.4 us of PE activity)


def _emit_core(nc, tc, pools, dram_io, core, variant="full"):
    import concourse.mybir as mybir

    f32 = mybir.dt.float32
    bf16 = mybir.dt.bfloat16
    apool, bpool, cpool, psum_pool, wpool = pools
    apack, bcat, cpart = dram_io["apack"], dram_io["bcat"], dram_io["cpart"]

    rows = sorted(BINS[core])
    mI = rows[0]
    bsuf = BTOT - BOFF[mI]  # resident B cols for this core

    if "empty" in variant:
        # Loop-overhead diagnostic: minimal body.
        z = wpool.tile([BLK, BLK], bf16, name="z", tag="warm", bufs=1)
        nc.gpsimd.memset(z[:], 0.0)
        return

    b_t = bpool.tile([BLK, bsuf], bf16, name="bres", tag="bres", bufs=1)
    a_t = apool.tile([BLK, NSLOT * BLK], bf16, name="a", tag="a", bufs=1)

    # HAM warmup: keep the PE busy while the first DMA chunks land so the
    # clock gate opens (4/8 -> 8/8) before real matmuls start.  Reads
    # uninitialized SBUF, writes a PSUM bank that row processing will
    # re-open with start=True.
    if "nomm" not in variant:
        wt = wpool.tile([BLK, BLK + PHASE], bf16, name="warm", tag="warm", bufs=1)
        nc.gpsimd.memset(wt[:], 0.0)
        wps = psum_pool.tile([BLK, PHASE], f32, name="wps", tag="ps7", bufs=1)
        for _ in range(N_WARM):
            nc.tensor.matmul(
                wps[:, :], wt[:, :BLK], wt[:, BLK:], start=True, stop=True
            )

    # B stream: ascending, 1 MB chunks, alternating the two HWDGE rings.
    if "nob" not in variant:
        qs = [nc.sync, nc.scalar]
        for qi, cur in enumerate(range(BOFF[mI], BTOT, BCHUNK)):
            end = min(cur + BCHUNK, BTOT)
            qs[qi % 2].dma_start(
                b_t[:, cur - BOFF[mI] : end - BOFF[mI]], bcat[:, cur:end]
            )
    else:
        # PE-only ablation: matmuls read a small memset window instead.
        nc.gpsimd.memset(b_t[:, :PHASE], 0.0)

    # A stream: per-row chunks on the SWDGE ring, in consumption order.
    for I in rows:
        base = ASLOT[core][I]
        for j0 in range(0, NB - I, ACHUNK):
            j1 = min(j0 + ACHUNK, NB - I)
            nc.gpsimd.dma_start(
                a_t[:, (base + j0) * BLK : (base + j1) * BLK],
                apack[:, (base + j0) * BLK : (base + j1) * BLK],
            )

    slot_row = {I: i for i, I in enumerate(rows)}

    def mm(ps_c, I, K, c, base):
        a_w = a_t[:, (base + K - I) * BLK : (base + K - I + 1) * BLK]
        cstart = max(PHASE * c, K * BLK)  # global col of this MM
        w = PHASE * (c + 1) - cstart
        boff = 0 if "nob" in variant else BOFF[K] - BOFF[mI] + cstart - K * BLK
        nc.tensor.matmul(
            ps_c[:, cstart - PHASE * c : PHASE],
            a_w,
            b_t[:, boff : boff + w],
            start=(K == I),
            stop=(K == min(4 * c + 3, NB - 1)),
        )

    pending = []

    def evict(ps_c, I, c, defer=False):
        if "noevict" in variant:
            return
        # Copy frees the PSUM bank immediately; the HBM store can be
        # deferred so it does not steal bandwidth from the B stream.
        coff0 = max(I * BLK - PHASE * c, 0)
        wv = PHASE - coff0
        ct = cpool.tile([BLK, PHASE], bf16, name=f"c_{I}_{c}", tag="cst", bufs=28)
        nc.vector.tensor_copy(ct[:, :wv], ps_c[:, coff0:PHASE])
        if defer:
            pending.append((I, c, ct, coff0, wv))
        else:
            r0 = slot_row[I] * BLK
            nc.gpsimd.dma_start(
                cpart[r0 : r0 + BLK, PHASE * c + coff0 : PHASE * (c + 1)], ct[:, :wv]
            )

    def flush_stores():
        # HWDGE rings only: SWDGE (gpsimd) descriptor generation can be
        # locked out of the shared SBUF port pair by the concurrent DVE
        # eviction copies of the post-chase bank-7 chains.
        sqs = [nc.sync, nc.scalar]
        for i, (I, c, ct, coff0, wv) in enumerate(pending):
            r0 = slot_row[I] * BLK
            sqs[i % 2].dma_start(
                cpart[r0 : r0 + BLK, PHASE * c + coff0 : PHASE * (c + 1)], ct[:, :wv]
            )
        pending.clear()

    if "nomm" in variant:
        return

    if "kmajor" in variant:
        # Fully serial reference schedule: each row K-major, rows in order.
        for I in rows:
            base = ASLOT[core][I]
            ps = {
                c: psum_pool.tile(
                    [BLK, PHASE], f32, name=f"ps_{I}_{c}", tag=f"ps{c}", bufs=1
                )
                for c in range(I // 4, NB // 4)
            }
            for K in range(I, NB):
                for c in range(K // 4, NB // 4):
                    mm(ps[c], I, K, c, base)
                if K % 4 == 3 and K // 4 >= I // 4:
                    evict(ps[K // 4], I, K // 4)
        return

    # Chase row (lowest I): K-major, consuming B strips as they arrive.
    # Bank c closes at K == 4c+3 and is evicted immediately; at that point
    # every strip a resident row's bank-c chain needs (K' <= 4c+3) is
    # already in SBUF, so those chains are emitted right there -- the PE
    # instruction order stays consumable under real DMA pacing and the
    # chase's DMA-wait gaps are filled with resident-row matmuls.
    I0 = rows[0]
    base0 = ASLOT[core][I0]
    ps = {
        c: psum_pool.tile([BLK, PHASE], f32, name=f"ps_{I0}_{c}", tag=f"ps{c}", bufs=1)
        for c in range(I0 // 4, NB // 4)
    }
    rot = 0  # physical-PSUM-bank rotation for resident chains
    for K in range(I0, NB):
        for c in range(K // 4, NB // 4):
            mm(ps[c], I0, K, c, base0)
        if K % 4 == 3:
            c = K // 4
            last = K == NB - 1
            evict(ps[c], I0, c, defer=not last)
            if last:
                # B stream is fully consumed: flush the deferred C stores
                # (they overlap the remaining bank-7 resident chains).
                flush_stores()
            kend = min(4 * c + 3, NB - 1)
            for R in rows[1:]:
                if R // 4 > c:
                    continue
                # Chase windows <= c are closed, so physical banks 0..c are
                # free; rotate chains across them so each chain's eviction
                # copy hides behind the next chains' matmuls instead of
                # stalling the PE on a same-bank WAR.
                b = rot % (c + 1)
                rot += 1
                ps_c = psum_pool.tile(
                    [BLK, PHASE], f32, name=f"ps_{R}_{c}", tag=f"ps{b}", bufs=1
                )
                for Kr in range(R, kend + 1):
                    mm(ps_c, R, Kr, c, base=ASLOT[core][R])
                evict(ps_c, R, c, defer=not last)
            if last:
                flush_stores()


def _build(mode=MODE, repeat=1, variant="full"):
    import concourse.mybir as mybir
    import concourse.tile as tile
    from concourse import bacc

    nc = bacc.Bacc(None, target_bir_lowering=False, debug=False)
    f32 = mybir.dt.float32
    bf16 = mybir.dt.bfloat16
    with tile.TileContext(nc) as tc:
        with (
            tc.tile_pool(name="dram", bufs=1, space="DRAM") as dram,
            tc.tile_pool(name="apool", bufs=1) as apool,
            tc.tile_pool(name="bpool", bufs=1) as bpool,
            tc.tile_pool(name="cpool", bufs=1) as cpool,
            tc.tile_pool(name="wpool", bufs=1) as wpool,
            tc.tile_pool(name="psum", bufs=1, space="PSUM") as psum_pool,
        ):
            dram_io = {
                "apack": dram.tile(
                    [BLK, NSLOT * BLK], bf16, kind="ExternalInput",
                    name="apack", uniquify=False,
                ),
                "bcat": dram.tile(
                    [BLK, BTOT], bf16, kind="ExternalInput",
                    name="bcat", uniquify=False,
                ),
                "cpart": dram.tile(
                    [MAXB * BLK, N], bf16, kind="ExternalOutput",
                    name="cpart", uniquify=False,
                ),
            }
            pid = nc.partition_id()
            pools = (apool, bpool, cpool, psum_pool, wpool)
            for c in range(N_CORES):
                cvar = variant
                if variant.startswith("solo"):
                    cvar = "full" if c == int(variant[4:]) else "empty"
                with tc.If(pid == c):
                    if repeat > 1:
                        with tc.For_i(
                            0, repeat, 1, hint_engines=tuple(mybir.ALL_ENGINES)
                        ):
                            _emit_core(nc, tc, pools, dram_io, c, cvar)
                    else:
                        _emit_core(nc, tc, pools, dram_io, c, cvar)
    nc.compile()
    return nc


_cached_nc = {}


def _get_nc(key=("full", 1)):
    if key not in _cached_nc:
        variant, repeat = key
        _cached_nc[key] = _build(repeat=repeat, variant=variant)
    return _cached_nc[key]


def _host_pack(A, B):
    import ml_dtypes

    bf16 = ml_dtypes.bfloat16
    AT = np.ascontiguousarray(A.T).astype(bf16)
    Bh = B.astype(bf16)
    bcat = np.empty((BLK, BTOT), dtype=bf16)
    for K in range(NB):
        bcat[:, BOFF[K] : BOFF[K] + (NB - K) * BLK] = Bh[K * BLK : (K + 1) * BLK, K * BLK :]
    apacks = []
    for c in range(N_CORES):
        ap = np.zeros((BLK, NSLOT * BLK), dtype=bf16)
        for I in sorted(BINS[c]):
            base = ASLOT[c][I]
            ap[:, base * BLK : (base + NB - I) * BLK] = np.concatenate(
                [AT[K * BLK : (K + 1) * BLK, I * BLK : (I + 1) * BLK] for K in range(I, NB)],
                axis=1,
            )
        apacks.append(ap)
    return apacks, bcat


LAST_RESULTS = None


def kernel(A, B):
    global LAST_RESULTS
    from concourse.bass_utils import run_bass_kernel_spmd

    A = np.asarray(A, dtype=np.float32)
    B = np.asarray(B, dtype=np.float32)
    nc = _get_nc()
    apacks, bcat = _host_pack(A, B)
    in_maps = [{"apack": apacks[c], "bcat": bcat} for c in range(N_CORES)]
    res = run_bass_kernel_spmd(nc, in_maps, core_ids=list(range(N_CORES)))
    LAST_RESULTS = res

    C = np.zeros((N, N), dtype=np.float32)
    for c in range(N_CORES):
        cp = res.results[c]["cpart"]
        for s, I in enumerate(sorted(BINS[c])):
            C[I * BLK : (I + 1) * BLK, I * BLK :] = cp[
                s * BLK : (s + 1) * BLK, I * BLK :
            ].astype(np.float32)
    return C
